# revision 16
# baseline (speedup 1.0000x reference)
"""Trainium2 Bass kernel for the lipsnet CustomModel problem.

Math: the reference computes, per sample,
    jac_norm = ||D3 W3 D2 W2 D1 W1||_F      (Di = diag(relu'(pi)))
    out = tanh(k_out * f_out / (jac_norm + 1e-4))
Key identity used here:  with G = W1 W1^T = L L^T (host eigen factorization),
    ||D3 W3 D2 W2 D1 W1||_F^2 = ||D3 W3 D2 W2 D1 L||_F^2
                              = sum_c || D3 W3 D2 (M_c @ d1) ||^2
where M_c[j,l] = W2[j,l] * L[l,c] are 85 host-precomputed stationary
matrices and d1/d2/d3 are the per-sample binary relu masks.  Every
per-sample 85x85x85 contraction becomes a stationary-weight matmul with
the mask tensor [85, S] as the moving operand, so the TensorEngine does
all the heavy lifting; the only full-size elementwise work per c is one
DVE mask-multiply and one ACT square.  The sum over c of squares is
accumulated on the TensorEngine itself via an identity-matmul into a
persistent PSUM tile.

Sharding: pure data parallel over the batch dim, 8 NeuronCores, weights
replicated.  kernel() takes FULL inputs and returns the FULL output.
"""

import os
from contextlib import ExitStack

import numpy as np

import concourse.bass as bass
import concourse.bacc as bacc
import concourse.mybir as mybir
import concourse.tile as tile

F32 = mybir.dt.float32
AF = mybir.ActivationFunctionType
OP = mybir.AluOpType

B = 8192
OBS = 64
ACTD = 16
H = 128
COMP = 85
KS = 32
NCORES = 8
S = B // NCORES        # 1024 samples per core
NB = S // 128          # 8 sample blocks of 128
CH = 512               # matmul moving-operand chunk (one PSUM bank of f32)
EPS = 1e-4

# name -> shape of every replicated (host-preprocessed) weight tensor
_WSPECS = {
    "ow1T": [OBS, H], "ob1": [H, 1], "ow2T": [H, H], "ob2": [H, 1],
    "aw1T": [ACTD, H], "ab1": [H, 1], "aw2T": [H, H], "ab2": [H, 1],
    "kw1Ta": [H, KS], "kw1Tb": [H, KS], "kb1": [KS, 1],
    "kw2T": [KS, KS // 2], "kb2": [KS // 2, 1],
    "kw3T": [KS // 2, 1], "kb3": [1, 1],
    "mw1Ta": [H, COMP], "mw1Tb": [H, COMP], "mb1": [COMP, 1],
    "mw2T": [COMP, COMP], "mb2": [COMP, 1],
    "mw3T": [COMP, COMP], "mb3": [COMP, 1],
    "mall": [COMP, COMP * COMP],
    "ones": [COMP, 1],
    "iden": [H, H],
}
# weights shipped in bf16 (J-loop matmul operands)
_BF16_W = {"mall", "mw3Tb"}
_WSPECS["mw3Tb"] = [COMP, COMP]


def host_prep(inputs):
    """Host-side weight preprocessing (pure numpy, all tiny)."""
    f = lambda a: np.ascontiguousarray(np.asarray(a, dtype=np.float32))
    W1, W2, W3 = f(inputs["mw1"]), f(inputs["mw2"]), f(inputs["mw3"])
    G = (W1 @ W1.T).astype(np.float64)
    lam, U = np.linalg.eigh(G)
    L = (U * np.sqrt(np.clip(lam, 0.0, None))).astype(np.float32)  # G = L L^T
    # mall[l, c*85+j] = W2[j, l] * L[l, c]   (stage-1 stationary lhsT per c)
    mall = (W2.T[:, None, :] * L[:, :, None]).reshape(COMP, COMP * COMP)
    w = {
        "ow1T": f(inputs["ow1"]).T, "ob1": f(inputs["ob1"]).reshape(H, 1),
        "ow2T": f(inputs["ow2"]).T, "ob2": f(inputs["ob2"]).reshape(H, 1),
        "aw1T": f(inputs["aw1"]).T, "ab1": f(inputs["ab1"]).reshape(H, 1),
        "aw2T": f(inputs["aw2"]).T, "ab2": f(inputs["ab2"]).reshape(H, 1),
        "kw1Ta": f(inputs["kw1"]).T[:H], "kw1Tb": f(inputs["kw1"]).T[H:],
        "kb1": f(inputs["kb1"]).reshape(KS, 1),
        "kw2T": f(inputs["kw2"]).T, "kb2": f(inputs["kb2"]).reshape(KS // 2, 1),
        "kw3T": f(inputs["kw3"]).T, "kb3": f(inputs["kb3"]).reshape(1, 1),
        "mw1Ta": W1.T[:H], "mw1Tb": W1.T[H:],
        "mb1": f(inputs["mb1"]).reshape(COMP, 1),
        "mw2T": W2.T, "mb2": f(inputs["mb2"]).reshape(COMP, 1),
        "mw3T": W3.T, "mb3": f(inputs["mb3"]).reshape(COMP, 1),
        "mall": mall,
        "mw3Tb": W3.T,
        "ones": np.ones((COMP, 1), np.float32),
        "iden": np.eye(H, dtype=np.float32),
    }
    import ml_dtypes
    out = {}
    for k, v in w.items():
        dt = ml_dtypes.bfloat16 if k in _BF16_W else np.float32
        out[k] = np.ascontiguousarray(np.asarray(v).astype(dt))
    return out


def build_nc():
    nc = bacc.Bacc()

    obs_d = nc.declare_dram_parameter("obs", [S, OBS], F32, isOutput=False)
    act_d = nc.declare_dram_parameter("action", [S, ACTD], F32, isOutput=False)
    BF16 = mybir.dt.bfloat16
    wd = {
        name: nc.declare_dram_parameter(
            name, shape, BF16 if name in _BF16_W else F32, isOutput=False)
        for name, shape in _WSPECS.items()
    }
    tick_d = nc.declare_dram_parameter("tick", [1, 1], F32, isOutput=False)
    out_d = nc.declare_dram_parameter("out", [S, COMP], F32, isOutput=True)

    with tile.TileContext(nc) as tc, ExitStack() as ctx:
        wp = ctx.enter_context(tc.tile_pool(name="weights", bufs=1))
        ap = ctx.enter_context(tc.tile_pool(name="acts", bufs=1))
        zp = ctx.enter_context(tc.tile_pool(name="zbuf", bufs=3))
        sqp = ctx.enter_context(tc.tile_pool(name="sqbuf", bufs=3))
        outp = ctx.enter_context(tc.tile_pool(name="outbuf", bufs=3))
        smp = ctx.enter_context(tc.tile_pool(name="small", bufs=16))
        psA = ctx.enter_context(tc.tile_pool(name="psA", bufs=2, space="PSUM"))
        psB = ctx.enter_context(tc.tile_pool(name="psB", bufs=1, space="PSUM"))
        psC = ctx.enter_context(tc.tile_pool(name="psC", bufs=1, space="PSUM"))

        # ---- load weights ----
        w = {}
        for name, shape in _WSPECS.items():
            w[name] = wp.tile(shape, BF16 if name in _BF16_W else F32,
                              tag=name, name=name)
            nc.sync.dma_start(w[name][:], wd[name][:])

        tick_sb = wp.tile([1, 1], F32, tag="tick_sb", name="tick_sb")
        nc.sync.dma_start(tick_sb[:], tick_d[:])

        # ---- load + transpose obs/action into [feat, S] layout ----
        obs_sb = ap.tile([128, NB, OBS], F32, tag="obs_sb")
        act_sb = ap.tile([128, NB, ACTD], F32, tag="act_sb")
        for nb in range(NB):
            nc.sync.dma_start(obs_sb[:, nb, :], obs_d[nb * 128:(nb + 1) * 128, :])
            nc.sync.dma_start(act_sb[:, nb, :], act_d[nb * 128:(nb + 1) * 128, :])
        # collapse the many DMA-queue semaphores into one barrier so no
        # matmul needs more than one sync wait (walrus S3_LW limit)
        tc.strict_bb_all_engine_barrier()

        obst = ap.tile([OBS, S], F32, tag="obst")
        actt = ap.tile([ACTD, S], F32, tag="actt")
        for nb in range(NB):
            pt = psA.tile([OBS, 128], F32, tag="a")
            nc.tensor.transpose(pt[:], obs_sb[:, nb, :], w["iden"][:])
            nc.scalar.copy(obst[:, nb * 128:(nb + 1) * 128], pt[:])
            pt2 = psA.tile([ACTD, 128], F32, tag="a")
            nc.tensor.transpose(pt2[:], act_sb[:, nb, :], w["iden"][:])
            nc.scalar.copy(actt[:, nb * 128:(nb + 1) * 128], pt2[:])

        # ---- forward layers ([feat, S], chunked matmuls + fused ACT) ----
        def layer(dst, dst_sl, terms, bias, func, p):
            # dst[dst_sl] = func(sum_i lhsT_i.T @ rhs_i + bias), chunked over S
            m = dst.shape[-1] if dst_sl is None else None
            for ch in range(S // CH):
                sl = slice(ch * CH, (ch + 1) * CH)
                pt = p.tile([terms[0][0].shape[-1], CH], F32, tag="a" if p is psA else "b")
                n = len(terms)
                for i, (lhsT, rhs) in enumerate(terms):
                    nc.tensor.matmul(pt[:], lhsT[:], rhs[:, sl],
                                     start=(i == 0), stop=(i == n - 1))
                dsl = dst[:, sl] if dst_sl is None else dst[dst_sl, sl]
                nc.scalar.activation(dsl, pt[:], func, bias=bias[:])

        oh1 = ap.tile([H, S], F32, tag="oh1")
        layer(oh1, None, [(w["ow1T"], obst)], w["ob1"], AF.Relu, psA)
        of = ap.tile([H, S], F32, tag="of")
        layer(of, None, [(w["ow2T"], oh1)], w["ob2"], AF.Relu, psA)
        ah1 = ap.tile([H, S], F32, tag="ah1")
        layer(ah1, None, [(w["aw1T"], actt)], w["ab1"], AF.Relu, psA)
        af = ap.tile([H, S], F32, tag="af")
        layer(af, None, [(w["aw2T"], ah1)], w["ab2"], AF.Relu, psA)

        k1 = ap.tile([KS, S], F32, tag="k1")
        layer(k1, None, [(w["kw1Ta"], of), (w["kw1Tb"], af)], w["kb1"], AF.Tanh, psA)
        k2 = ap.tile([KS // 2, S], F32, tag="k2")
        layer(k2, None, [(w["kw2T"], k1)], w["kb2"], AF.Tanh, psA)

        # k_out = softplus(kw3 @ k2 + kb3) = ln(1 + exp(.)) via Exp then Ln(x+1)
        kexp = ap.tile([1, S], F32, tag="kexp")
        layer(kexp, None, [(w["kw3T"], k2)], w["kb3"], AF.Exp, psA)
        kout = ap.tile([1, S], F32, tag="kout")
        nc.scalar.activation(kout[:], kexp[:], AF.Ln, bias=1.0)

        h1 = ap.tile([COMP, S], F32, tag="h1")
        layer(h1, None, [(w["mw1Ta"], of), (w["mw1Tb"], af)], w["mb1"], AF.Relu, psA)
        d1 = ap.tile([COMP, S], BF16, tag="d1")
        nc.vector.tensor_scalar(out=d1[:], in0=h1[:], scalar1=0.0, scalar2=None,
                                op0=OP.is_gt)
        h2 = ap.tile([COMP, S], F32, tag="h2")
        layer(h2, None, [(w["mw2T"], h1)], w["mb2"], AF.Relu, psA)
        d2 = ap.tile([COMP, S], F32, tag="d2")
        nc.vector.tensor_scalar(out=d2[:], in0=h2[:], scalar1=0.0, scalar2=None,
                                op0=OP.is_gt)
        fout = ap.tile([COMP, S], F32, tag="fout")
        layer(fout, None, [(w["mw3T"], h2)], w["mb3"], AF.Relu, psA)
        d3 = ap.tile([COMP, S], F32, tag="d3")
        nc.vector.tensor_scalar(out=d3[:], in0=fout[:], scalar1=0.0,
                                scalar2=None, op0=OP.is_gt)

        # ---- Jacobian-norm loop over the 85 columns of L ----
        # bf16 identity for the accumulate-matmul (fp32 matmuls lower to
        # HI/LO pairs that break inside an interleaved accumulation group)
        idenb = wp.tile([COMP, COMP], BF16, tag="idenb", name="idenb")
        nc.vector.tensor_copy(idenb[:], w["iden"][:COMP, :COMP])
        accp = psC.tile([COMP, S], F32, tag="c")   # persistent PSUM accumulator
        for c in range(COMP):
            py = psA.tile([COMP, S], F32, tag="a")
            for ch in range(S // CH):
                sl = slice(ch * CH, (ch + 1) * CH)
                nc.tensor.matmul(py[:, sl], w["mall"][:, c * COMP:(c + 1) * COMP],
                                 d1[:, sl], start=True, stop=True)
            z = zp.tile([COMP, S], BF16, tag="z")
            nc.vector.tensor_tensor(z[:], py[:], d2[:], OP.mult)
            pr = psB.tile([COMP, S], F32, tag="b")
            for ch in range(S // CH):
                sl = slice(ch * CH, (ch + 1) * CH)
                nc.tensor.matmul(pr[:, sl], w["mw3Tb"][:], z[:, sl],
                                 start=True, stop=True)
            sq = sqp.tile([COMP, S], BF16, tag="sq")
            nc.scalar.square(sq[:], pr[:])
            acc_grp = os.environ.get("K_NOACC") != "1"
            for ch in range(S // CH):
                sl = slice(ch * CH, (ch + 1) * CH)
                nc.tensor.matmul(accp[:, sl], idenb[:], sq[:, sl],
                                 start=(c == 0) if acc_grp else True,
                                 stop=(c == COMP - 1) if acc_grp else True,
                                 skip_group_check=True)

        # ---- finale: jn2 = ones^T (d3 * acc); out = tanh(kout*fout/(sqrt+eps)) ----
        am = zp.tile([COMP, S], F32, tag="am")
        nc.vector.tensor_tensor(am[:], accp[:], d3[:], OP.mult)
        pj = psA.tile([1, S], F32, tag="a")
        for ch in range(S // CH):
            sl = slice(ch * CH, (ch + 1) * CH)
            nc.tensor.matmul(pj[:, sl], w["ones"][:], am[:, sl],
                             start=True, stop=True)
        jn2 = ap.tile([1, S], F32, tag="jn2")
        nc.scalar.copy(jn2[:], pj[:])

        tc.strict_bb_all_engine_barrier()

        for nb in range(NB):
            sl = slice(nb * 128, (nb + 1) * 128)
            pt = psA.tile([128, COMP + 2], F32, tag="a")
            nc.tensor.transpose(pt[:, 0:COMP], fout[:, sl], w["iden"][:COMP, :COMP])
            nc.tensor.transpose(pt[:, COMP:COMP + 1], jn2[:, sl], w["iden"][:1, :1])
            nc.tensor.transpose(pt[:, COMP + 1:COMP + 2], kout[:, sl],
                                w["iden"][:1, :1])
            den = smp.tile([128, 1], F32, tag="den")
            nc.scalar.activation(den[:], pt[:, COMP:COMP + 1], AF.Sqrt)
            den2 = smp.tile([128, 1], F32, tag="den2")
            nc.vector.tensor_scalar_add(den2[:], den[:], EPS)
            rec = smp.tile([128, 1], F32, tag="rec")
            nc.vector.reciprocal(rec[:], den2[:])
            scl = smp.tile([128, 1], F32, tag="scl")
            nc.vector.tensor_tensor(scl[:], rec[:], pt[:, COMP + 1:COMP + 2], OP.mult)
            ot = outp.tile([128, COMP], F32, tag="ot")
            nc.scalar.activation(ot[:], pt[:, 0:COMP], AF.Tanh, scale=scl[:])
            nc.sync.dma_start(out_d[sl, :], ot[:])

    return nc


_NC = None


def _get_nc():
    global _NC
    if _NC is None:
        _NC = build_nc()
        _NC.finalize()
    return _NC


def make_in_maps(inputs):
    w = host_prep(inputs)
    obs = np.ascontiguousarray(np.asarray(inputs["obs"], np.float32))
    act = np.ascontiguousarray(np.asarray(inputs["action"], np.float32))
    in_maps = []
    for i in range(NCORES):
        m = dict(w)
        m["obs"] = np.ascontiguousarray(obs[i * S:(i + 1) * S])
        m["action"] = np.ascontiguousarray(act[i * S:(i + 1) * S])
        m["tick"] = np.zeros((1, 1), np.float32)
        in_maps.append(m)
    return in_maps


def kernel(**inputs):
    from concourse.bass_utils import run_bass_kernel_spmd

    nc = _get_nc()
    in_maps = make_in_maps(inputs)
    res = run_bass_kernel_spmd(nc, in_maps, core_ids=list(range(NCORES)))
    return np.concatenate([r["out"] for r in res.results], axis=0)
